# revision 32
# baseline (speedup 1.0000x reference)
"""Trainium2 Bass kernel for nn_AbsoluteNeuralLayer.

Reference computation:
    classical = x @ classical_weights + classical_biases          # [B, DOUT]
    probs[j]  = |scan of circulant "rotations" applied to s0|[0]^2
    out       = tanh(classical + probs[None, :])

Key simplification: the scan state s0 is a constant vector, and every step
maps a constant vector to a constant vector scaled by cos(angle)
(s_new[i] = cos*s - sin*s + sin*s = cos*s elementwise).  Hence
    probs[j] = (prod_{t<48} cos(ang[j, t]))^2 / DIN
with ang[j, 3*d+g] = absolute_weights[d, j, g] for g < 3.

Sharding (8 cores): batch split 4 ways x dout split 2 ways.  Each core
computes out[1024 batch rows, 1024 dout cols] as tanh(x_s @ W_s + bias_s +
probs_s) with dout on PSUM partitions and batch on the moving free dim
(fp32r matmuls: fp32 accuracy at 16-bit PE streaming rate), accumulating
over K=2048 in 16 k-tiles.  probs+bias are computed once per core on
ACT/DVE (tiny) and applied as the per-partition bias of the Tanh
activation that drains PSUM.  Outputs are written transposed and
un-transposed on the host during the gather.

Schedule (8 PSUM banks → 8 concurrent accumulation groups):
  pass A: all 8 n-tiles x batch-chunk 0, k-outer; DMA-paced while W + x0
          stream in (tapered chunks so the first matmul starts ~3 us in).
  pass B: batch-chunk 1 in n sub-phases (4+2+2); B's first matmuls only
          wait for pass A's first epilogues (same banks), so the PE never
          idles long enough for the HAM to re-throttle, and only the last
          sub-phase's epilogues land in the kernel tail.
All inputs are host-packed into SBUF layout so every DMA is a single
contiguous-per-partition transfer (~420 GB/s sustained).
"""

import math

import numpy as np

import concourse.bacc as bacc
import concourse.mybir as mybir
from concourse.tile import TileContext
from concourse.bass_utils import run_bass_kernel_spmd

B, DIN, DOUT, DEPTH = 4096, 2048, 2048, 16
NCORES = 8
BB, DB = 4, 2            # batch blocks x dout blocks (BB*DB == NCORES)
MB, NB = B // BB, DOUT // DB   # per-core batch rows (1024) / dout cols (1024)
KT = DIN // 128          # 16 contraction tiles
NT = NB // 128           # 8 dout tiles
MCH = 512                # batch chunk = one PSUM bank of fp32
MC = MB // MCH           # 2 chunks
NANG = 3 * DEPTH         # 48 angles per output column

A_CHUNKS = [1, 1, 2, 2, 2, 2, 2, 2, 1, 1]   # pass-A stream: small first chunks so
                                            # the first matmul starts ~2.5 us
                                            # earlier, uniform middle pacing (each
                                            # DMA-deficit stall < HAM window),
                                            # tapered tail
B_CHUNKS = [2, 2, 4, 4, 4]               # x1 stream
B_SUBS = [4, 2, 2]                       # pass-B n-tile sub-phases (tail stagger)
WARMUP_MMS = 50                          # PE warmup matmuls (HAM un-throttle):
                                         # ~5.4 us of PE activity guarantees a
                                         # full HAM SHORT window and bridges to
                                         # the first input chunk (~14 us)

F32 = mybir.dt.float32
F32R = mybir.dt.float32r
AF = mybir.ActivationFunctionType

_NC_CACHE = None


def _chunk_offsets(chunks):
    off, out = 0, []
    for c in chunks:
        out.append((off, c))
        off += c
    return out


def _build():
    nc = bacc.Bacc("TRN2", target_bir_lowering=False, debug=False, num_devices=NCORES)
    # host-packed SBUF layouts:
    #   wb [p, k*NB + n]          = W[128k+p, n]
    #   xb [p, (u*KT + k)*MCH+m]  = x[u*MCH + m, 128k+p]   (u = m-chunk)
    wb = nc.dram_tensor("wb", [128, KT * NB], F32R, kind="ExternalInput")
    xb = nc.dram_tensor("xb", [128, MC * KT * MCH], F32R, kind="ExternalInput")
    ang = nc.dram_tensor("ang", [128, NT * NANG], F32, kind="ExternalInput")
    bias = nc.dram_tensor("bias", [128, NT], F32, kind="ExternalInput")
    outT = nc.dram_tensor("outT", [NB, MB], F32, kind="ExternalOutput")

    with TileContext(nc) as tc:
        with (
            tc.tile_pool(name="big", bufs=1) as big,
            tc.tile_pool(name="small", bufs=1) as small,
            tc.tile_pool(name="outp", bufs=8) as outp,
            tc.tile_pool(name="psum", bufs=1, space="PSUM") as psump,
        ):
            # ang + bias first: tiny (200 KB) but the probs chain gates the
            # first epilogue, so they must not queue behind the bulk stream
            ang_sb = small.tile([128, NT * NANG], F32, tag="ang")
            nc.sync.dma_start(out=ang_sb, in_=ang[:, :])
            bias_sb = small.tile([128, NT], F32, tag="bias")
            nc.sync.dma_start(out=bias_sb, in_=bias[:, :])

            # ---- pass-A stream: W (full) + x chunk 0, k-chunked, tapered ----
            wg = [None] * KT   # (tile, col offset) per k
            xs = [[None] * KT for _ in range(MC)]
            for ci, (k0, kn) in enumerate(_chunk_offsets(A_CHUNKS)):
                wt = big.tile([128, kn * NB], F32R, tag=f"w{ci}", name=f"w{ci}")
                nc.sync.dma_start(out=wt, in_=wb[:, k0 * NB:(k0 + kn) * NB])
                for i in range(kn):
                    wg[k0 + i] = (wt, i * NB)
                xt = big.tile([128, kn * MCH], F32R, tag=f"x0_{ci}", name=f"x0_{ci}")
                nc.sync.dma_start(out=xt, in_=xb[:, k0 * MCH:(k0 + kn) * MCH])
                for i in range(kn):
                    xs[0][k0 + i] = (xt, i * MCH)

            # ---- probs + bias compute (tiny, ACT/DVE) ----
            halfpi = small.tile([128, 1], F32, tag="halfpi")
            nc.any.memset(halfpi, math.pi / 2)
            cos_sb = small.tile([128, NT * NANG], F32, tag="cos")
            nc.scalar.activation(cos_sb, ang_sb, AF.Sin, bias=halfpi)

            def v3(t):
                return t.rearrange("p (a b) -> p a b", a=NT)

            t24 = small.tile([128, NT * 24], F32, tag="t24")
            nc.vector.tensor_mul(v3(t24), v3(cos_sb)[:, :, 0:24], v3(cos_sb)[:, :, 24:48])
            t12 = small.tile([128, NT * 12], F32, tag="t12")
            nc.vector.tensor_mul(v3(t12), v3(t24)[:, :, 0:12], v3(t24)[:, :, 12:24])
            t6 = small.tile([128, NT * 6], F32, tag="t6")
            nc.vector.tensor_mul(v3(t6), v3(t12)[:, :, 0:6], v3(t12)[:, :, 6:12])
            t3 = small.tile([128, NT * 3], F32, tag="t3")
            nc.vector.tensor_mul(v3(t3), v3(t6)[:, :, 0:3], v3(t6)[:, :, 3:6])
            t1 = small.tile([128, NT], F32, tag="t1")
            nc.vector.tensor_mul(v3(t1), v3(t3)[:, :, 0:1], v3(t3)[:, :, 1:2])
            nc.vector.tensor_mul(v3(t1), v3(t1), v3(t3)[:, :, 2:3])
            sq = small.tile([128, NT], F32, tag="sq")
            nc.vector.tensor_mul(sq, t1, t1)
            nc.vector.tensor_scalar_mul(sq, sq, 1.0 / DIN)
            btot = small.tile([128, NT], F32, tag="btot")
            nc.vector.tensor_add(btot, sq, bias_sb)

            def mm_w(k, n):
                wt, off = wg[k]
                return wt[:, off + 128 * n:off + 128 * (n + 1)]

            def mm_x(u, k):
                xt, off = xs[u][k]
                return xt[:, off:off + MCH]

            def epilogue(n, ps_tile, u, dma_eng):
                # ACT on scalar; out DMA issued from whichever HWDGE ring is
                # idle at that point (scalar during the input stream, sync
                # once the input stream has drained)
                o = outp.tile([128, MCH], F32, tag="o", name=f"o{n}_{u}")
                nc.scalar.activation(o, ps_tile, AF.Tanh, bias=btot[:, n:n + 1])
                dma_eng.dma_start(
                    out=outT[128 * n:128 * (n + 1), u * MCH:(u + 1) * MCH], in_=o
                )

            # ---- pass A: m-chunk 0, k-outer over 8 PSUM groups ----
            psA = [
                psump.tile([128, MCH], F32, tag=f"ps{n}", name=f"psA{n}")
                for n in range(NT)
            ]
            # PE warmup: ~40 dependency-free matmuls into psA[0] flip the HAM
            # clock gate to 8/8 before the first real matmul's data lands, so
            # the stream starts at 2.4 GHz instead of 1.2 GHz.
            warm = small.tile([128, 128], mybir.dt.bfloat16, tag="warm")
            nc.any.memset(warm, 0.0)
            for i in range(WARMUP_MMS):
                nc.tensor.matmul(psA[0][:, 0:128], warm, warm, start=True, stop=True)
            for k in range(KT):
                for n in range(NT):
                    nc.tensor.matmul(
                        psA[n], mm_w(k, n), mm_x(0, k),
                        start=(k == 0), stop=(k == KT - 1),
                    )

            # x chunk 1 stream (issued here so the ring stays fed behind the
            # pass-A bytes without competing with them)
            for ci, (k0, kn) in enumerate(_chunk_offsets(B_CHUNKS)):
                xt = big.tile([128, kn * MCH], F32R, tag=f"x1_{ci}", name=f"x1_{ci}")
                nc.sync.dma_start(
                    out=xt, in_=xb[:, (KT + k0) * MCH:(KT + k0 + kn) * MCH]
                )
                for i in range(kn):
                    xs[1][k0 + i] = (xt, i * MCH)

            # pass A epilogues (ACT) — free banks in n order for pass B
            for n in range(NT):
                epilogue(n, psA[n], 0, nc.scalar)

            # ---- pass B: m-chunk 1, n sub-phases for staggered tail ----
            n0 = 0
            for nsub in B_SUBS:
                psB = [
                    psump.tile(
                        [128, MCH], F32, tag=f"ps{n0 + t}", name=f"psB{n0 + t}"
                    )
                    for t in range(nsub)
                ]
                for k in range(KT):
                    for t in range(nsub):
                        nc.tensor.matmul(
                            psB[t], mm_w(k, n0 + t), mm_x(1, k),
                            start=(k == 0), stop=(k == KT - 1),
                        )
                for t in range(nsub):
                    epilogue(n0 + t, psB[t], 1, nc.sync)
                n0 += nsub

    nc.compile()
    return nc


def _get_nc():
    global _NC_CACHE
    if _NC_CACHE is None:
        _NC_CACHE = _build()
    return _NC_CACHE


def _in_map_for_core(core, x, absolute_weights, classical_weights, classical_biases):
    i, j = core % BB, core // BB
    rows = slice(i * MB, (i + 1) * MB)
    cols = slice(j * NB, (j + 1) * NB)
    # wb[p, k*NB + n] = W[128k+p, n]
    wbm = np.ascontiguousarray(
        classical_weights[:, cols].reshape(KT, 128, NB).transpose(1, 0, 2).reshape(128, KT * NB)
    )
    # xb[p, (u*KT + k)*MCH + m] = x[rows][u*MCH+m, 128k+p]
    xsT = x[rows, :].T                                        # [DIN, MB] view
    xr = xsT.reshape(KT, 128, MC, MCH)                        # [k, p, u, m]
    xbm = np.ascontiguousarray(xr.transpose(1, 2, 0, 3).reshape(128, MC * KT * MCH))
    # ang[j_local, 3*d+g] = absolute_weights[d, j, g]
    angj = np.transpose(absolute_weights[:, cols, :3], (1, 0, 2)).reshape(NB, NANG)
    ang_sb = np.ascontiguousarray(
        angj.reshape(NT, 128, NANG).transpose(1, 0, 2).reshape(128, NT * NANG)
    )
    bias_sb = np.ascontiguousarray(classical_biases[cols].reshape(NT, 128).T)
    return {
        "wb": wbm.astype(np.float32, copy=False),
        "xb": xbm.astype(np.float32, copy=False),
        "ang": ang_sb.astype(np.float32, copy=False),
        "bias": bias_sb.astype(np.float32, copy=False),
    }


def kernel(x, absolute_weights, classical_weights, classical_biases, **_ignored):
    x = np.asarray(x, dtype=np.float32)
    absolute_weights = np.asarray(absolute_weights, dtype=np.float32)
    classical_weights = np.asarray(classical_weights, dtype=np.float32)
    classical_biases = np.asarray(classical_biases, dtype=np.float32)

    nc = _get_nc()
    in_maps = [
        _in_map_for_core(c, x, absolute_weights, classical_weights, classical_biases)
        for c in range(NCORES)
    ]
    res = run_bass_kernel_spmd(nc, in_maps, list(range(NCORES)))

    out = np.empty((B, DOUT), np.float32)
    for c in range(NCORES):
        i, j = c % BB, c // BB
        out[i * MB:(i + 1) * MB, j * NB:(j + 1) * NB] = res.results[c]["outT"].T
    return out


# revision 34
# speedup vs baseline: 1.0259x; 1.0259x over previous
"""Trainium2 Bass kernel for nn_AbsoluteNeuralLayer.

Reference computation:
    classical = x @ classical_weights + classical_biases          # [B, DOUT]
    probs[j]  = |scan of circulant "rotations" applied to s0|[0]^2
    out       = tanh(classical + probs[None, :])

Key simplification: the scan state s0 is a constant vector, and every step
maps a constant vector to a constant vector scaled by cos(angle)
(s_new[i] = cos*s - sin*s + sin*s = cos*s elementwise).  Hence
    probs[j] = (prod_{t<48} cos(ang[j, t]))^2 / DIN
with ang[j, 3*d+g] = absolute_weights[d, j, g] for g < 3.

Sharding (8 cores): batch split 4 ways x dout split 2 ways.  Each core
computes out[1024 batch rows, 1024 dout cols] as tanh(x_s @ W_s + bias_s +
probs_s) with dout on PSUM partitions and batch on the moving free dim
(fp32r matmuls: fp32 accuracy at 16-bit PE streaming rate), accumulating
over K=2048 in 16 k-tiles.  probs+bias are computed once per core on
ACT/DVE (tiny) and applied as the per-partition bias of the Tanh
activation that drains PSUM.  Outputs are written transposed and
un-transposed on the host during the gather.

Schedule (8 PSUM banks → 8 concurrent accumulation groups):
  pass A: all 8 n-tiles x batch-chunk 0, k-outer; DMA-paced while W + x0
          stream in (tapered chunks so the first matmul starts ~3 us in).
  pass B: batch-chunk 1 in n sub-phases (4+2+2); B's first matmuls only
          wait for pass A's first epilogues (same banks), so the PE never
          idles long enough for the HAM to re-throttle, and only the last
          sub-phase's epilogues land in the kernel tail.
All inputs are host-packed into SBUF layout so every DMA is a single
contiguous-per-partition transfer (~420 GB/s sustained).
"""

import math

import numpy as np

import concourse.bacc as bacc
import concourse.mybir as mybir
from concourse.tile import TileContext
from concourse.bass_utils import run_bass_kernel_spmd

B, DIN, DOUT, DEPTH = 4096, 2048, 2048, 16
NCORES = 8
BB, DB = 4, 2            # batch blocks x dout blocks (BB*DB == NCORES)
MB, NB = B // BB, DOUT // DB   # per-core batch rows (1024) / dout cols (1024)
KT = DIN // 128          # 16 contraction tiles
NT = NB // 128           # 8 dout tiles
MCH = 512                # batch chunk = one PSUM bank of fp32
MC = MB // MCH           # 2 chunks
NANG = 3 * DEPTH         # 48 angles per output column

A_CHUNKS = [1, 1, 2, 2, 2, 2, 2, 2, 1, 1]   # pass-A stream: small first chunks so
                                            # the first matmul starts ~2.5 us
                                            # earlier, uniform middle pacing (each
                                            # DMA-deficit stall < HAM window),
                                            # tapered tail
B_CHUNKS = [2, 2, 4, 4, 4]               # x1 stream
B_SUBS = [4, 2, 1, 1]                    # pass-B n-tile sub-phases; final sub is a
                                         # single group so only one epilogue+out
                                         # serializes after the last matmul
WARMUP_MMS = 40                          # PE warmup matmuls (HAM un-throttle)

F32 = mybir.dt.float32
F32R = mybir.dt.float32r
AF = mybir.ActivationFunctionType

_NC_CACHE = None


def _chunk_offsets(chunks):
    off, out = 0, []
    for c in chunks:
        out.append((off, c))
        off += c
    return out


def _build():
    nc = bacc.Bacc("TRN2", target_bir_lowering=False, debug=False, num_devices=NCORES)
    # host-packed SBUF layouts:
    #   wb [p, k*NB + n]          = W[128k+p, n]
    #   xb [p, (u*KT + k)*MCH+m]  = x[u*MCH + m, 128k+p]   (u = m-chunk)
    wb = nc.dram_tensor("wb", [128, KT * NB], F32R, kind="ExternalInput")
    xb = nc.dram_tensor("xb", [128, MC * KT * MCH], F32R, kind="ExternalInput")
    ang = nc.dram_tensor("ang", [128, NT * NANG], F32, kind="ExternalInput")
    bias = nc.dram_tensor("bias", [128, NT], F32, kind="ExternalInput")
    outT = nc.dram_tensor("outT", [NB, MB], F32, kind="ExternalOutput")

    with TileContext(nc) as tc:
        with (
            tc.tile_pool(name="big", bufs=1) as big,
            tc.tile_pool(name="small", bufs=1) as small,
            tc.tile_pool(name="outp", bufs=8) as outp,
            tc.tile_pool(name="psum", bufs=1, space="PSUM") as psump,
        ):
            # ang + bias first: tiny (200 KB) but the probs chain gates the
            # first epilogue, so they must not queue behind the bulk stream
            ang_sb = small.tile([128, NT * NANG], F32, tag="ang")
            nc.sync.dma_start(out=ang_sb, in_=ang[:, :])
            bias_sb = small.tile([128, NT], F32, tag="bias")
            nc.sync.dma_start(out=bias_sb, in_=bias[:, :])

            # ---- pass-A stream: W (full) + x chunk 0, k-chunked, tapered ----
            wg = [None] * KT   # (tile, col offset) per k
            xs = [[None] * KT for _ in range(MC)]
            for ci, (k0, kn) in enumerate(_chunk_offsets(A_CHUNKS)):
                wt = big.tile([128, kn * NB], F32R, tag=f"w{ci}", name=f"w{ci}")
                nc.sync.dma_start(out=wt, in_=wb[:, k0 * NB:(k0 + kn) * NB])
                for i in range(kn):
                    wg[k0 + i] = (wt, i * NB)
                xt = big.tile([128, kn * MCH], F32R, tag=f"x0_{ci}", name=f"x0_{ci}")
                nc.sync.dma_start(out=xt, in_=xb[:, k0 * MCH:(k0 + kn) * MCH])
                for i in range(kn):
                    xs[0][k0 + i] = (xt, i * MCH)

            # ---- probs + bias compute (tiny, ACT/DVE) ----
            halfpi = small.tile([128, 1], F32, tag="halfpi")
            nc.any.memset(halfpi, math.pi / 2)
            cos_sb = small.tile([128, NT * NANG], F32, tag="cos")
            nc.scalar.activation(cos_sb, ang_sb, AF.Sin, bias=halfpi)

            def v3(t):
                return t.rearrange("p (a b) -> p a b", a=NT)

            t24 = small.tile([128, NT * 24], F32, tag="t24")
            nc.vector.tensor_mul(v3(t24), v3(cos_sb)[:, :, 0:24], v3(cos_sb)[:, :, 24:48])
            t12 = small.tile([128, NT * 12], F32, tag="t12")
            nc.vector.tensor_mul(v3(t12), v3(t24)[:, :, 0:12], v3(t24)[:, :, 12:24])
            t6 = small.tile([128, NT * 6], F32, tag="t6")
            nc.vector.tensor_mul(v3(t6), v3(t12)[:, :, 0:6], v3(t12)[:, :, 6:12])
            t3 = small.tile([128, NT * 3], F32, tag="t3")
            nc.vector.tensor_mul(v3(t3), v3(t6)[:, :, 0:3], v3(t6)[:, :, 3:6])
            t1 = small.tile([128, NT], F32, tag="t1")
            nc.vector.tensor_mul(v3(t1), v3(t3)[:, :, 0:1], v3(t3)[:, :, 1:2])
            nc.vector.tensor_mul(v3(t1), v3(t1), v3(t3)[:, :, 2:3])
            sq = small.tile([128, NT], F32, tag="sq")
            nc.vector.tensor_mul(sq, t1, t1)
            nc.vector.tensor_scalar_mul(sq, sq, 1.0 / DIN)
            btot = small.tile([128, NT], F32, tag="btot")
            nc.vector.tensor_add(btot, sq, bias_sb)

            def mm_w(k, n):
                wt, off = wg[k]
                return wt[:, off + 128 * n:off + 128 * (n + 1)]

            def mm_x(u, k):
                xt, off = xs[u][k]
                return xt[:, off:off + MCH]

            def epilogue(n, ps_tile, u, dma_eng):
                # ACT on scalar; out DMA issued from whichever HWDGE ring is
                # idle at that point (scalar during the input stream, sync
                # once the input stream has drained)
                o = outp.tile([128, MCH], F32, tag="o", name=f"o{n}_{u}")
                nc.scalar.activation(o, ps_tile, AF.Tanh, bias=btot[:, n:n + 1])
                dma_eng.dma_start(
                    out=outT[128 * n:128 * (n + 1), u * MCH:(u + 1) * MCH], in_=o
                )

            # ---- pass A: m-chunk 0, k-outer over 8 PSUM groups ----
            psA = [
                psump.tile([128, MCH], F32, tag=f"ps{n}", name=f"psA{n}")
                for n in range(NT)
            ]
            # PE warmup: ~40 dependency-free matmuls into psA[0] flip the HAM
            # clock gate to 8/8 before the first real matmul's data lands, so
            # the stream starts at 2.4 GHz instead of 1.2 GHz.
            warm = small.tile([128, 128], mybir.dt.bfloat16, tag="warm")
            nc.any.memset(warm, 0.0)
            for i in range(WARMUP_MMS):
                nc.tensor.matmul(psA[0][:, 0:128], warm, warm, start=True, stop=True)
            for k in range(KT):
                for n in range(NT):
                    nc.tensor.matmul(
                        psA[n], mm_w(k, n), mm_x(0, k),
                        start=(k == 0), stop=(k == KT - 1),
                    )

            # x chunk 1 stream (issued here so the ring stays fed behind the
            # pass-A bytes without competing with them)
            for ci, (k0, kn) in enumerate(_chunk_offsets(B_CHUNKS)):
                xt = big.tile([128, kn * MCH], F32R, tag=f"x1_{ci}", name=f"x1_{ci}")
                nc.sync.dma_start(
                    out=xt, in_=xb[:, (KT + k0) * MCH:(KT + k0 + kn) * MCH]
                )
                for i in range(kn):
                    xs[1][k0 + i] = (xt, i * MCH)

            # pass A epilogues (ACT) — free banks in n order for pass B
            for n in range(NT):
                epilogue(n, psA[n], 0, nc.scalar)

            # ---- pass B: m-chunk 1, n sub-phases for staggered tail ----
            n0 = 0
            for nsub in B_SUBS:
                psB = [
                    psump.tile(
                        [128, MCH], F32, tag=f"ps{n0 + t}", name=f"psB{n0 + t}"
                    )
                    for t in range(nsub)
                ]
                for k in range(KT):
                    for t in range(nsub):
                        nc.tensor.matmul(
                            psB[t], mm_w(k, n0 + t), mm_x(1, k),
                            start=(k == 0), stop=(k == KT - 1),
                        )
                for t in range(nsub):
                    epilogue(n0 + t, psB[t], 1, nc.sync)
                n0 += nsub

    nc.compile()
    return nc


def _get_nc():
    global _NC_CACHE
    if _NC_CACHE is None:
        _NC_CACHE = _build()
    return _NC_CACHE


def _in_map_for_core(core, x, absolute_weights, classical_weights, classical_biases):
    i, j = core % BB, core // BB
    rows = slice(i * MB, (i + 1) * MB)
    cols = slice(j * NB, (j + 1) * NB)
    # wb[p, k*NB + n] = W[128k+p, n]
    wbm = np.ascontiguousarray(
        classical_weights[:, cols].reshape(KT, 128, NB).transpose(1, 0, 2).reshape(128, KT * NB)
    )
    # xb[p, (u*KT + k)*MCH + m] = x[rows][u*MCH+m, 128k+p]
    xsT = x[rows, :].T                                        # [DIN, MB] view
    xr = xsT.reshape(KT, 128, MC, MCH)                        # [k, p, u, m]
    xbm = np.ascontiguousarray(xr.transpose(1, 2, 0, 3).reshape(128, MC * KT * MCH))
    # ang[j_local, 3*d+g] = absolute_weights[d, j, g]
    angj = np.transpose(absolute_weights[:, cols, :3], (1, 0, 2)).reshape(NB, NANG)
    ang_sb = np.ascontiguousarray(
        angj.reshape(NT, 128, NANG).transpose(1, 0, 2).reshape(128, NT * NANG)
    )
    bias_sb = np.ascontiguousarray(classical_biases[cols].reshape(NT, 128).T)
    return {
        "wb": wbm.astype(np.float32, copy=False),
        "xb": xbm.astype(np.float32, copy=False),
        "ang": ang_sb.astype(np.float32, copy=False),
        "bias": bias_sb.astype(np.float32, copy=False),
    }


def kernel(x, absolute_weights, classical_weights, classical_biases, **_ignored):
    x = np.asarray(x, dtype=np.float32)
    absolute_weights = np.asarray(absolute_weights, dtype=np.float32)
    classical_weights = np.asarray(classical_weights, dtype=np.float32)
    classical_biases = np.asarray(classical_biases, dtype=np.float32)

    nc = _get_nc()
    in_maps = [
        _in_map_for_core(c, x, absolute_weights, classical_weights, classical_biases)
        for c in range(NCORES)
    ]
    res = run_bass_kernel_spmd(nc, in_maps, list(range(NCORES)))

    out = np.empty((B, DOUT), np.float32)
    for c in range(NCORES):
        i, j = c % BB, c // BB
        out[i * MB:(i + 1) * MB, j * NB:(j + 1) * NB] = res.results[c]["outT"].T
    return out


# revision 35
# speedup vs baseline: 1.1021x; 1.0742x over previous
"""Trainium2 Bass kernel for nn_AbsoluteNeuralLayer.

Reference computation:
    classical = x @ classical_weights + classical_biases          # [B, DOUT]
    probs[j]  = |scan of circulant "rotations" applied to s0|[0]^2
    out       = tanh(classical + probs[None, :])

Key simplification: the scan state s0 is a constant vector, and every step
maps a constant vector to a constant vector scaled by cos(angle)
(s_new[i] = cos*s - sin*s + sin*s = cos*s elementwise).  Hence
    probs[j] = (prod_{t<48} cos(ang[j, t]))^2 / DIN
with ang[j, 3*d+g] = absolute_weights[d, j, g] for g < 3.

Sharding (8 cores): batch split 4 ways x dout split 2 ways.  Each core
computes out[1024 batch rows, 1024 dout cols] as tanh(x_s @ W_s + bias_s +
probs_s) with dout on PSUM partitions and batch on the moving free dim
(fp32r matmuls: fp32 accuracy at 16-bit PE streaming rate), accumulating
over K=2048 in 16 k-tiles.  probs+bias are computed once per core on
ACT/DVE (tiny) and applied as the per-partition bias of the Tanh
activation that drains PSUM.  Outputs are written transposed and
un-transposed on the host during the gather.

Schedule (8 PSUM banks → 8 concurrent accumulation groups):
  pass A: all 8 n-tiles x batch-chunk 0, k-outer; DMA-paced while W + x0
          stream in (tapered chunks so the first matmul starts ~3 us in).
  pass B: batch-chunk 1 in n sub-phases (4+2+2); B's first matmuls only
          wait for pass A's first epilogues (same banks), so the PE never
          idles long enough for the HAM to re-throttle, and only the last
          sub-phase's epilogues land in the kernel tail.
All inputs are host-packed into SBUF layout so every DMA is a single
contiguous-per-partition transfer (~420 GB/s sustained).
"""

import math

import numpy as np

import concourse.bacc as bacc
import concourse.mybir as mybir
from concourse.tile import TileContext
from concourse.bass_utils import run_bass_kernel_spmd

B, DIN, DOUT, DEPTH = 4096, 2048, 2048, 16
NCORES = 8
BB, DB = 4, 2            # batch blocks x dout blocks (BB*DB == NCORES)
MB, NB = B // BB, DOUT // DB   # per-core batch rows (1024) / dout cols (1024)
KT = DIN // 128          # 16 contraction tiles
NT = NB // 128           # 8 dout tiles
MCH = 512                # batch chunk = one PSUM bank of fp32
MC = MB // MCH           # 2 chunks
NANG = 3 * DEPTH         # 48 angles per output column

A_CHUNKS = [1, 1, 2, 2, 2, 2, 2, 2, 1, 1]   # pass-A stream: small first chunks so
                                            # the first matmul starts ~2.5 us
                                            # earlier, uniform middle pacing (each
                                            # DMA-deficit stall < HAM window),
                                            # tapered tail
B_CHUNKS = [2, 2, 4, 4, 4]               # x1 stream
B_SUBS = [4, 2, 2]                       # pass-B n-tile sub-phases (tail stagger)
WARMUP_MMS = 40                          # PE warmup matmuls (HAM un-throttle)

F32 = mybir.dt.float32
F32R = mybir.dt.float32r
AF = mybir.ActivationFunctionType

_NC_CACHE = None


def _chunk_offsets(chunks):
    off, out = 0, []
    for c in chunks:
        out.append((off, c))
        off += c
    return out


def _build():
    nc = bacc.Bacc("TRN2", target_bir_lowering=False, debug=False, num_devices=NCORES)
    # host-packed SBUF layouts:
    #   wb [p, k*NB + n]          = W[128k+p, n]
    #   xb [p, (u*KT + k)*MCH+m]  = x[u*MCH + m, 128k+p]   (u = m-chunk)
    wb = nc.dram_tensor("wb", [128, KT * NB], F32R, kind="ExternalInput")
    xb = nc.dram_tensor("xb", [128, MC * KT * MCH], F32R, kind="ExternalInput")
    ang = nc.dram_tensor("ang", [128, NT * NANG], F32, kind="ExternalInput")
    bias = nc.dram_tensor("bias", [128, NT], F32, kind="ExternalInput")
    outT = nc.dram_tensor("outT", [NB, MB], F32, kind="ExternalOutput")

    with TileContext(nc) as tc:
        with (
            tc.tile_pool(name="big", bufs=1) as big,
            tc.tile_pool(name="small", bufs=1) as small,
            tc.tile_pool(name="outp", bufs=8) as outp,
            tc.tile_pool(name="psum", bufs=1, space="PSUM") as psump,
        ):
            # ang + bias first: tiny (200 KB) but the probs chain gates the
            # first epilogue, so they must not queue behind the bulk stream
            ang_sb = small.tile([128, NT * NANG], F32, tag="ang")
            nc.sync.dma_start(out=ang_sb, in_=ang[:, :])
            bias_sb = small.tile([128, NT], F32, tag="bias")
            nc.sync.dma_start(out=bias_sb, in_=bias[:, :])

            # ---- pass-A stream: W (full) + x chunk 0, k-chunked, tapered ----
            wg = [None] * KT   # (tile, col offset) per k
            xs = [[None] * KT for _ in range(MC)]
            for ci, (k0, kn) in enumerate(_chunk_offsets(A_CHUNKS)):
                wt = big.tile([128, kn * NB], F32R, tag=f"w{ci}", name=f"w{ci}")
                nc.sync.dma_start(out=wt, in_=wb[:, k0 * NB:(k0 + kn) * NB])
                for i in range(kn):
                    wg[k0 + i] = (wt, i * NB)
                xt = big.tile([128, kn * MCH], F32R, tag=f"x0_{ci}", name=f"x0_{ci}")
                nc.sync.dma_start(out=xt, in_=xb[:, k0 * MCH:(k0 + kn) * MCH])
                for i in range(kn):
                    xs[0][k0 + i] = (xt, i * MCH)

            # ---- probs + bias compute (tiny, ACT/DVE) ----
            halfpi = small.tile([128, 1], F32, tag="halfpi")
            nc.any.memset(halfpi, math.pi / 2)
            cos_sb = small.tile([128, NT * NANG], F32, tag="cos")
            nc.scalar.activation(cos_sb, ang_sb, AF.Sin, bias=halfpi)

            def v3(t):
                return t.rearrange("p (a b) -> p a b", a=NT)

            t24 = small.tile([128, NT * 24], F32, tag="t24")
            nc.vector.tensor_mul(v3(t24), v3(cos_sb)[:, :, 0:24], v3(cos_sb)[:, :, 24:48])
            t12 = small.tile([128, NT * 12], F32, tag="t12")
            nc.vector.tensor_mul(v3(t12), v3(t24)[:, :, 0:12], v3(t24)[:, :, 12:24])
            t6 = small.tile([128, NT * 6], F32, tag="t6")
            nc.vector.tensor_mul(v3(t6), v3(t12)[:, :, 0:6], v3(t12)[:, :, 6:12])
            t3 = small.tile([128, NT * 3], F32, tag="t3")
            nc.vector.tensor_mul(v3(t3), v3(t6)[:, :, 0:3], v3(t6)[:, :, 3:6])
            t1 = small.tile([128, NT], F32, tag="t1")
            nc.vector.tensor_mul(v3(t1), v3(t3)[:, :, 0:1], v3(t3)[:, :, 1:2])
            nc.vector.tensor_mul(v3(t1), v3(t1), v3(t3)[:, :, 2:3])
            sq = small.tile([128, NT], F32, tag="sq")
            nc.vector.tensor_mul(sq, t1, t1)
            nc.vector.tensor_scalar_mul(sq, sq, 1.0 / DIN)
            btot = small.tile([128, NT], F32, tag="btot")
            nc.vector.tensor_add(btot, sq, bias_sb)

            def mm_w(k, n):
                wt, off = wg[k]
                return wt[:, off + 128 * n:off + 128 * (n + 1)]

            def mm_x(u, k):
                xt, off = xs[u][k]
                return xt[:, off:off + MCH]

            def epilogue(n, ps_tile, u, dma_eng):
                # ACT on scalar; out DMA issued from whichever HWDGE ring is
                # idle at that point (scalar during the input stream, sync
                # once the input stream has drained)
                o = outp.tile([128, MCH], F32, tag="o", name=f"o{n}_{u}")
                nc.scalar.activation(o, ps_tile, AF.Tanh, bias=btot[:, n:n + 1])
                dma_eng.dma_start(
                    out=outT[128 * n:128 * (n + 1), u * MCH:(u + 1) * MCH], in_=o
                )

            # ---- pass A: m-chunk 0, k-outer over 8 PSUM groups ----
            psA = [
                psump.tile([128, MCH], F32, tag=f"ps{n}", name=f"psA{n}")
                for n in range(NT)
            ]
            # PE warmup: ~40 dependency-free matmuls into psA[0] flip the HAM
            # clock gate to 8/8 before the first real matmul's data lands, so
            # the stream starts at 2.4 GHz instead of 1.2 GHz.
            warm = small.tile([128, 128], mybir.dt.bfloat16, tag="warm")
            nc.any.memset(warm, 0.0)
            for i in range(WARMUP_MMS):
                nc.tensor.matmul(psA[0][:, 0:128], warm, warm, start=True, stop=True)
            for k in range(KT):
                for n in range(NT):
                    nc.tensor.matmul(
                        psA[n], mm_w(k, n), mm_x(0, k),
                        start=(k == 0), stop=(k == KT - 1),
                    )

            # x chunk 1 stream (issued here so the ring stays fed behind the
            # pass-A bytes without competing with them)
            for ci, (k0, kn) in enumerate(_chunk_offsets(B_CHUNKS)):
                xt = big.tile([128, kn * MCH], F32R, tag=f"x1_{ci}", name=f"x1_{ci}")
                nc.sync.dma_start(
                    out=xt, in_=xb[:, (KT + k0) * MCH:(KT + k0 + kn) * MCH]
                )
                for i in range(kn):
                    xs[1][k0 + i] = (xt, i * MCH)

            # pass A epilogues (ACT) — free banks in n order for pass B
            for n in range(NT):
                epilogue(n, psA[n], 0, nc.scalar)

            # ---- pass B: m-chunk 1, n sub-phases for staggered tail ----
            n0 = 0
            for nsub in B_SUBS:
                psB = [
                    psump.tile(
                        [128, MCH], F32, tag=f"ps{n0 + t}", name=f"psB{n0 + t}"
                    )
                    for t in range(nsub)
                ]
                for k in range(KT):
                    for t in range(nsub):
                        nc.tensor.matmul(
                            psB[t], mm_w(k, n0 + t), mm_x(1, k),
                            start=(k == 0), stop=(k == KT - 1),
                        )
                for t in range(nsub):
                    epilogue(n0 + t, psB[t], 1, nc.sync)
                n0 += nsub

    nc.compile()
    return nc


def _get_nc():
    global _NC_CACHE
    if _NC_CACHE is None:
        _NC_CACHE = _build()
    return _NC_CACHE


def _in_map_for_core(core, x, absolute_weights, classical_weights, classical_biases):
    i, j = core % BB, core // BB
    rows = slice(i * MB, (i + 1) * MB)
    cols = slice(j * NB, (j + 1) * NB)
    # wb[p, k*NB + n] = W[128k+p, n]
    wbm = np.ascontiguousarray(
        classical_weights[:, cols].reshape(KT, 128, NB).transpose(1, 0, 2).reshape(128, KT * NB)
    )
    # xb[p, (u*KT + k)*MCH + m] = x[rows][u*MCH+m, 128k+p]
    xsT = x[rows, :].T                                        # [DIN, MB] view
    xr = xsT.reshape(KT, 128, MC, MCH)                        # [k, p, u, m]
    xbm = np.ascontiguousarray(xr.transpose(1, 2, 0, 3).reshape(128, MC * KT * MCH))
    # ang[j_local, 3*d+g] = absolute_weights[d, j, g]
    angj = np.transpose(absolute_weights[:, cols, :3], (1, 0, 2)).reshape(NB, NANG)
    ang_sb = np.ascontiguousarray(
        angj.reshape(NT, 128, NANG).transpose(1, 0, 2).reshape(128, NT * NANG)
    )
    bias_sb = np.ascontiguousarray(classical_biases[cols].reshape(NT, 128).T)
    return {
        "wb": wbm.astype(np.float32, copy=False),
        "xb": xbm.astype(np.float32, copy=False),
        "ang": ang_sb.astype(np.float32, copy=False),
        "bias": bias_sb.astype(np.float32, copy=False),
    }


def kernel(x, absolute_weights, classical_weights, classical_biases, **_ignored):
    x = np.asarray(x, dtype=np.float32)
    absolute_weights = np.asarray(absolute_weights, dtype=np.float32)
    classical_weights = np.asarray(classical_weights, dtype=np.float32)
    classical_biases = np.asarray(classical_biases, dtype=np.float32)

    nc = _get_nc()
    in_maps = [
        _in_map_for_core(c, x, absolute_weights, classical_weights, classical_biases)
        for c in range(NCORES)
    ]
    res = run_bass_kernel_spmd(nc, in_maps, list(range(NCORES)))

    out = np.empty((B, DOUT), np.float32)
    for c in range(NCORES):
        i, j = c % BB, c // BB
        out[i * MB:(i + 1) * MB, j * NB:(j + 1) * NB] = res.results[c]["outT"].T
    return out
